# revision 3
# baseline (speedup 1.0000x reference)
"""Trainium2 Bass kernel for nn_EquivariantMatrix (group conv over Z16 x Z16).

Math: out[b,f,h] = sum_{i,s} kernel[f,i,s] * x[b,i,h (-) s] + bias[f]
(2D circular convolution over the 16x16 translation group; the reference's
536MB expanded-kernel tensor is never materialized).

Sharding: data-parallel over batch, 2 batches per core on 8 cores.

Per-core device work (all fp32 data, float32r matmul mode):
  - xe[t][p=(s2off*32+i), col=(g1*32+h2*2+bl)] = x[b0+bl, i, g1, (h2-(4t+s2off))%16]
    (4 SBUF tiles 128x512, host-prepared)
  - kt[pp][p=(s2off*32+i), col=(t*128+s1off*64+f)] = kernel[f,i,2pp+s1off,4t+s2off]
    (8 SBUF tiles 128x512, host-prepared, replicated on all cores)
  - one psum tile (128,512) accumulates:
      bias rank-1 matmul (start=True), then for pp in 0..7, t in 0..3 the two
      wraparound pieces of matmul(kt[pp][:,t], xe[t]) with column offsets
      aligning even s1=2pp to output h1; odd s1=2pp+1 lands rotated by one h1.
  - drain: out[f, (h1,h2,bl)] = psum[f, .] + psum[64+f, (h1-1)%16 cols]
"""

import numpy as np

L1 = L2 = 16
S = 256
I = 32
F = 64
B = 16
NCORES = 8
BPC = 2  # batches per core

_cache = {}


def _build_nc():
    from concourse import bacc
    import concourse.tile as tile
    import concourse.mybir as mybir

    f32 = mybir.dt.float32
    f32r = mybir.dt.float32r

    nc = bacc.Bacc(None, target_bir_lowering=False, debug=False)
    xe_d = nc.dram_tensor("xe", (4, 128, 512), f32r, kind="ExternalInput")
    kt_d = nc.dram_tensor("kt", (8, 128, 512), f32r, kind="ExternalInput")
    blhs_d = nc.dram_tensor("blhs", (1, 128), f32r, kind="ExternalInput")
    ones_d = nc.dram_tensor("ones", (1, 512), f32r, kind="ExternalInput")
    out_d = nc.dram_tensor("out", (64, 512), f32, kind="ExternalOutput")

    with tile.TileContext(nc) as tc:
        with (
            tc.tile_pool(name="data", bufs=1) as pool,
            tc.tile_pool(name="ps", bufs=1, space="PSUM") as pspool,
        ):
            xe_t = [pool.tile([128, 512], f32r, name=f"xe{t}", tag=f"xe{t}") for t in range(4)]
            kt_t = [pool.tile([128, 512], f32r, name=f"kt{p}", tag=f"kt{p}") for p in range(8)]
            blhs = pool.tile([1, 128], f32r, tag="blhs")
            ones = pool.tile([1, 512], f32r, tag="ones")
            out_t = pool.tile([64, 512], f32, tag="out")
            psum = pspool.tile([128, 512], f32, tag="psum")

            for t in range(4):
                nc.sync.dma_start(xe_t[t][:], xe_d[t])
            nc.sync.dma_start(blhs[:], blhs_d[:])
            nc.sync.dma_start(ones[:], ones_d[:])
            for p in range(8):
                nc.sync.dma_start(kt_t[p][:], kt_d[p])

            # bias as rank-1 update; start=True initializes the whole psum tile
            nc.tensor.matmul(psum[:], blhs[:], ones[:], start=True, stop=False,
                             skip_group_check=True)

            for pp in range(8):
                s1e = 2 * pp
                for t in range(4):
                    lhsT = kt_t[pp][:, t * 128:(t + 1) * 128]
                    nA = (16 - s1e) * 32
                    last = (pp == 7 and t == 3)
                    # piece A: g1 in [0,16-s1e) -> psum cols [s1e*32, 512)
                    nc.tensor.matmul(psum[:, s1e * 32:512], lhsT,
                                     xe_t[t][:, 0:nA], start=False,
                                     stop=(last and s1e == 0),
                                     skip_group_check=True)
                    # piece B: g1 in [16-s1e,16) -> psum cols [0, s1e*32)
                    if s1e > 0:
                        nc.tensor.matmul(psum[:, 0:s1e * 32], lhsT,
                                         xe_t[t][:, nA:512], start=False,
                                         stop=last, skip_group_check=True)

            # drain: even-s1 half + odd-s1 half rotated by +1 in h1.
            # (only one tensor-op input may come from PSUM, so bounce the
            # rotated odd half through SBUF first)
            tmp = pool.tile([64, 512], f32, tag="tmp")
            nc.vector.tensor_copy(tmp[:, 32:512], psum[64:128, 0:480])
            nc.vector.tensor_copy(tmp[:, 0:32], psum[64:128, 480:512])
            nc.vector.tensor_add(out_t[:], psum[0:64, :], tmp[:])

            nc.sync.dma_start(out_d[:], out_t[:])

    nc.finalize()
    return nc


def _host_prep_kt(kern):
    # kt[pp, p=(s2off*32+i), t*128 + s1off*64 + f] = kern[f, i, 2pp+s1off, 4t+s2off]
    k4 = kern.reshape(F, I, 8, 2, 4, 4)          # f, i, pp, s1off, t, s2off
    kt = k4.transpose(2, 4, 5, 1, 3, 0)          # pp, t, s2off, i, s1off, f
    kt = kt.reshape(8, 4, 128, 128)              # pp, t, p, (s1off*64+f)
    kt = kt.transpose(0, 2, 1, 3)                # pp, p, t, col
    return np.ascontiguousarray(kt.reshape(8, 128, 512), dtype=np.float32)


def _host_prep_xe(xc):
    # xe[t, s2off*32+i, g1*32+h2*2+bl] = xc[bl, i, g1, (h2-(4t+s2off))%16]
    x4 = xc.reshape(BPC, I, L1, L2)
    xe = np.empty((4, 128, 512), np.float32)
    for t in range(4):
        for s2off in range(4):
            s2 = 4 * t + s2off
            shifted = np.roll(x4, s2, axis=3)        # bl, i, g1, h2
            shifted = shifted.transpose(1, 2, 3, 0)  # i, g1, h2, bl
            xe[t, s2off * 32:(s2off + 1) * 32, :] = shifted.reshape(I, 512)
    return xe


def kernel(x, kernel, bias, product_table):
    from concourse.bass_utils import run_bass_kernel_spmd

    x = np.ascontiguousarray(np.asarray(x), dtype=np.float32)
    kern = np.ascontiguousarray(np.asarray(kernel), dtype=np.float32)
    bias = np.asarray(bias, dtype=np.float32)

    if _cache.get("nc") is None:
        _cache["nc"] = _build_nc()
    nc = _cache["nc"]

    kt = _host_prep_kt(kern)
    # bias rank-1: psum[m, :] += 0.5*bias[m%64] * ones  (each half gets it once,
    # and the odd half is rotated-added onto the even half -> total = bias)
    blhs = np.concatenate([bias, bias]).reshape(1, 128).astype(np.float32) * 0.5
    ones = np.ones((1, 512), np.float32)

    in_maps = []
    for c in range(NCORES):
        in_maps.append({
            "xe": _host_prep_xe(x[BPC * c:BPC * (c + 1)]),
            "kt": kt,
            "blhs": blhs,
            "ones": ones,
        })

    res = run_bass_kernel_spmd(nc, in_maps, list(range(NCORES)))

    out = np.empty((B, F, S), np.float32)
    for c in range(NCORES):
        o = res.results[c]["out"]                    # (64, 512)
        o = o.reshape(F, L1, L2, BPC).transpose(3, 0, 1, 2)
        out[BPC * c:BPC * (c + 1)] = o.reshape(BPC, F, S)
    return out


# revision 4
# speedup vs baseline: 1.1315x; 1.1315x over previous
"""Trainium2 Bass kernel for nn_EquivariantMatrix (group conv over Z16 x Z16).

Math: out[b,f,h] = sum_{i,s} kernel[f,i,s] * x[b,i,h (-) s] + bias[f]
(2D circular convolution over the 16x16 translation group; the reference's
536MB expanded-kernel tensor is never materialized).

Sharding: data-parallel over batch, 2 batches per core on 8 cores.

Per-core device plan (fp32 data, float32r matmul mode):
  - xe2[t][p=(s2off*32+i), col=(g1pad*32+h2*2+bl)], g1pad in [0,32) doubled:
    value x[b0+bl, i, g1pad%16, (h2-(4t+s2off))%16]. DMA brings the g1pad<16
    half (host-prepared); a DVE copy duplicates it into g1pad>=16.
  - ktt[t][p=(s2off*32+i), col=(pp*128+s1off*64+f)] = kernel[f,i,2pp+s1off,4t+s2off]
  - one psum tile (128,512) accumulates a bias rank-1 matmul (start=True) then
    for t in 0..3, pp in 0..7 a single N=512 matmul whose rhs column offset
    ((16-2pp)%16)*32 into the doubled g1pad axis aligns even s1=2pp with the
    output h1; odd s1=2pp+1 lands rotated by one h1.
  - drain: out[f,(h1,h2,bl)] = psum[f,.] + psum[64+f, (h1-1)%16 cols] (+bias
    via the rank-1 matmul, halved since both halves receive it).
"""

import numpy as np

L1 = L2 = 16
S = 256
I = 32
F = 64
B = 16
NCORES = 8
BPC = 2  # batches per core

_cache = {}


def _build_nc():
    from concourse import bacc
    import concourse.tile as tile
    import concourse.mybir as mybir

    f32 = mybir.dt.float32
    f32r = mybir.dt.float32r

    nc = bacc.Bacc(None, target_bir_lowering=False, debug=False)
    xe_d = nc.dram_tensor("xe", (4, 128, 512), f32r, kind="ExternalInput")
    kt_d = nc.dram_tensor("kt", (4, 128, 1024), f32r, kind="ExternalInput")
    misc_d = nc.dram_tensor("misc", (1, 640), f32r, kind="ExternalInput")
    out_d = nc.dram_tensor("out", (64, 512), f32, kind="ExternalOutput")

    with tile.TileContext(nc) as tc:
        with (
            tc.tile_pool(name="data", bufs=1) as pool,
            tc.tile_pool(name="ps", bufs=1, space="PSUM") as pspool,
        ):
            xe_t = [pool.tile([128, 1024], f32r, name=f"xe{t}", tag=f"xe{t}")
                    for t in range(4)]
            kt_t = [pool.tile([128, 1024], f32r, name=f"kt{t}", tag=f"kt{t}")
                    for t in range(4)]
            misc = pool.tile([1, 640], f32r, tag="misc")
            out_t = pool.tile([64, 512], f32, tag="out")
            tmp = pool.tile([64, 512], f32, tag="tmp")
            psum = pspool.tile([128, 512], f32, tag="psum")

            # DMAs: xe from scalar's queue, ktt from sync's queue (parallel
            # issue), interleaved in use order (t-major).
            nc.sync.dma_start(misc[:], misc_d[:])
            for t in range(4):
                nc.scalar.dma_start(xe_t[t][:, 0:512], xe_d[t])
                nc.sync.dma_start(kt_t[t][:], kt_d[t])
                # duplicate into the padded g1pad half (read by pp>=1 only)
                nc.vector.tensor_copy(xe_t[t][:, 512:1024], xe_t[t][:, 0:512])

            blhs = misc[:, 0:128]
            ones = misc[:, 128:640]

            # bias as rank-1 update; start=True initializes the whole psum tile
            nc.tensor.matmul(psum[:], blhs, ones, start=True, stop=False,
                             skip_group_check=True)

            for t in range(4):
                for pp in range(8):
                    s1e = 2 * pp
                    off = ((16 - s1e) % 16) * 32  # pp=0 -> 0 (unpadded half)
                    lhsT = kt_t[t][:, pp * 128:(pp + 1) * 128]
                    nc.tensor.matmul(psum[:], lhsT,
                                     xe_t[t][:, off:off + 512],
                                     start=False,
                                     stop=(t == 3 and pp == 7),
                                     skip_group_check=True)

            # drain: even-s1 half + odd-s1 half rotated by +1 in h1.
            # (only one tensor-op input may come from PSUM, so bounce the
            # rotated odd half through SBUF first)
            nc.vector.tensor_copy(tmp[:, 32:512], psum[64:128, 0:480])
            nc.vector.tensor_copy(tmp[:, 0:32], psum[64:128, 480:512])
            nc.vector.tensor_add(out_t[:], psum[0:64, :], tmp[:])

            nc.sync.dma_start(out_d[:], out_t[:])

    nc.finalize()
    return nc


def _host_prep_kt(kern):
    # ktt[t, p=(s2off*32+i), pp*128 + s1off*64 + f] = kern[f, i, 2pp+s1off, 4t+s2off]
    k4 = kern.reshape(F, I, 8, 2, 4, 4)          # f, i, pp, s1off, t, s2off
    kt = k4.transpose(4, 5, 1, 2, 3, 0)          # t, s2off, i, pp, s1off, f
    return np.ascontiguousarray(kt.reshape(4, 128, 1024), dtype=np.float32)


def _host_prep_xe(xc):
    # xe[t, s2off*32+i, g1*32+h2*2+bl] = xc[bl, i, g1, (h2-(4t+s2off))%16]
    x4 = xc.reshape(BPC, I, L1, L2)
    xe = np.empty((4, 128, 512), np.float32)
    for t in range(4):
        for s2off in range(4):
            s2 = 4 * t + s2off
            shifted = np.roll(x4, s2, axis=3)        # bl, i, g1, h2
            shifted = shifted.transpose(1, 2, 3, 0)  # i, g1, h2, bl
            xe[t, s2off * 32:(s2off + 1) * 32, :] = shifted.reshape(I, 512)
    return xe


def _make_in_maps(x, kern, bias):
    kt = _host_prep_kt(kern)
    misc = np.zeros((1, 640), np.float32)
    # bias rank-1: psum[m, :] += 0.5*bias[m%64]; each half receives it once and
    # the odd half is rotated-added onto the even half -> total = bias
    misc[0, 0:128] = np.concatenate([bias, bias]) * 0.5
    misc[0, 128:640] = 1.0
    return [{
        "xe": _host_prep_xe(x[BPC * c:BPC * (c + 1)]),
        "kt": kt,
        "misc": misc,
    } for c in range(NCORES)]


def _assemble(results):
    out = np.empty((B, F, S), np.float32)
    for c in range(NCORES):
        o = results[c]["out"]                        # (64, 512)
        o = o.reshape(F, L1, L2, BPC).transpose(3, 0, 1, 2)
        out[BPC * c:BPC * (c + 1)] = o.reshape(BPC, F, S)
    return out


def kernel(x, kernel, bias, product_table):
    from concourse.bass_utils import run_bass_kernel_spmd

    x = np.ascontiguousarray(np.asarray(x), dtype=np.float32)
    kern = np.ascontiguousarray(np.asarray(kernel), dtype=np.float32)
    bias = np.asarray(bias, dtype=np.float32)

    if _cache.get("nc") is None:
        _cache["nc"] = _build_nc()

    in_maps = _make_in_maps(x, kern, bias)
    res = run_bass_kernel_spmd(_cache["nc"], in_maps, list(range(NCORES)))
    return _assemble(res.results)
